# revision 15
# baseline (speedup 1.0000x reference)
"""CTC loss kernel for Trainium2 (8 NeuronCores, batch-parallel).

Strategy (v3)
-------------
Batch B=64 sharded 8 samples/core. pred is cast to fp8(e4m3) on the HOST, so
the device streams 8.5 MB/core instead of 34 MB (HBM read is the roofline:
measured ~125-170 GB/s/core regardless of queue mix, so fewer bytes win).
Per core, ten [128, 6625] tiles (partition p = b*16 + r) stream through:

  1. DMA in, alternating the two HWDGE queues (sync/scalar); the first two
     tile PAIRS are split into sample-halves issued on BOTH queues in
     parallel so the DP pipeline starts ~35us earlier. Tiles 0-4 load
     time-ascending; tiles 9-5 load TIME-REVERSED via a negative-stride
     DRAM access pattern (r = 15 - t_local).
  2. ScalarE: Exp fp8->f32 with fused per-row accumulate. The softmax
     denominators s are NOT applied on device - they are written out and
     folded back on the host (log-domain, f64). A 31-column zero pad is
     memset next to the exp output.
  3. GPSIMD ap_gather picks 64 columns per row: the extended-label class
     columns, with per-sample band masking (state > 2*target_len) baked in
     by pointing masked states at the zero pad column. Tiles 0-4 gather
     forward states; tiles 9-5 gather MIRRORED states for the backward DP.
  4. Regroup (b*16+r, s) -> (b, r, s) via DRAM scratch (cast to bf16 on the
     way out) into a [16, 16, 64] bf16 tile: rows 0-7 = forward p, rows
     8-15 = backward p-tilde.

The CTC recursion is split in half: alpha runs forward over t=0..79 (rows
0-7) while beta runs backward over t=159..80 (rows 8-15, state-mirrored so
its shifts match alpha's). Both advance in the SAME VectorE instructions
([16, 51] bf16 tiles), halving the serial chain to 80 steps:

  new[s] = (cur[s] + cur[s-1] + skip[s]*cur[s-2]) * p[t, s]

Renormalization every 8 steps is POSTPONED: the sum is computed on GPSIMD
(off the VectorE critical path) and applied as a single tensor-scalar
divide fused after the next step's multiply; the final step renormalizes
in-chain. Host epilogue: L = sum_s alpha_79[s] * (beta_80[s] + beta_80[s+1]
+ skip[s+2]*beta_80[s+2]), NLL = -(log L + sum log c - sum_t log s_t),
zero-infinity, length division, batch mean (f64, 64 samples).
"""

from contextlib import ExitStack

import numpy as np
import ml_dtypes

import concourse.bass as bass
import concourse.tile as tile
from concourse import bacc, mybir
from concourse.ap import AP
from concourse.bass_utils import run_bass_kernel_spmd

N_CORES = 8
B = 64
T = 160
C = 6625
L = 25
S = 2 * L + 1           # 51 extended states
BPC = B // N_CORES      # 8 samples per core
TBLK = 16               # time steps per streamed tile
NBLK = T // TBLK        # 10 tiles per core
GC = 64                 # gather columns (51 states padded to 64)
CP = C + 31             # exp tile free size incl. zero pad columns
ZCOL = C                # index of the (memset) zero column
ROWS = 16               # 0-7 fwd alpha, 8-15 bwd beta (mirrored)
NPH = 5                 # DP phases (80 steps / 16)
NNORM = 10              # renorm sums at j = 7, 15, ..., 79
NSPLIT = 4              # ORDER positions whose DMA is split across both queues
ORDER = [0, 9, 1, 8, 2, 7, 3, 6, 4, 5]

FP = mybir.dt.float32
BF = mybir.dt.bfloat16
F8 = mybir.dt.float8e4
ADD = mybir.AluOpType.add
DIV = mybir.AluOpType.divide


def build_nc() -> bass.Bass:
    nc = bacc.Bacc("TRN2", target_bir_lowering=False, debug=False,
                   num_devices=N_CORES)
    pred = nc.dram_tensor("pred", [BPC, T, C], F8, kind="ExternalInput")
    idxf = nc.dram_tensor("idxf", [128, GC // 16], mybir.dt.int16,
                          kind="ExternalInput")
    idxb = nc.dram_tensor("idxb", [128, GC // 16], mybir.dt.int16,
                          kind="ExternalInput")
    maskv = nc.dram_tensor("maskv", [ROWS, GC], BF, kind="ExternalInput")
    initm = nc.dram_tensor("initm", [ROWS, GC], BF, kind="ExternalInput")
    out_alpha = nc.dram_tensor("out_alpha", [ROWS, GC], BF, kind="ExternalOutput")
    out_c = nc.dram_tensor("out_c", [ROWS, NNORM], FP, kind="ExternalOutput")
    out_s = nc.dram_tensor("out_s", [128, NBLK], FP, kind="ExternalOutput")

    with tile.TileContext(nc) as tc, ExitStack() as ctx:
        pred_pool = ctx.enter_context(tc.tile_pool(name="pred_pool", bufs=3))
        exp_pool = ctx.enter_context(tc.tile_pool(name="exp_pool", bufs=3))
        small = ctx.enter_context(tc.tile_pool(name="small", bufs=3))
        pdp_pool = ctx.enter_context(tc.tile_pool(name="pdp_pool", bufs=3))
        dram_pool = ctx.enter_context(
            tc.tile_pool(name="pscr_pool", bufs=4, space="DRAM"))

        def single(shape, dtype, name):
            t, free = tc.tile(shape, dtype, name=name)
            ctx.callback(free)
            return t

        idxf_sb = single([128, GC // 16], mybir.dt.int16, "idxf_sb")
        idxb_sb = single([128, GC // 16], mybir.dt.int16, "idxb_sb")
        maskv_sb = single([ROWS, GC], BF, "maskv_sb")
        initm_sb = single([ROWS, GC], BF, "initm_sb")
        # ping/pong alpha with 2 guard columns each: ping states at 2..52,
        # pong states at 66..116; guards stay zero forever.
        alpha = single([ROWS, 128], BF, "alpha")
        cbuf = single([ROWS, NNORM], FP, "cbuf")
        rcn = single([ROWS, 1], FP, "rcn")
        sacc = single([128, NBLK], FP, "sacc")

        # constants go on the gpsimd queue so the first pred DMA leads sync
        nc.gpsimd.dma_start(out=idxf_sb[:, :], in_=idxf[:, :])
        nc.gpsimd.dma_start(out=idxb_sb[:, :], in_=idxb[:, :])
        nc.gpsimd.dma_start(out=maskv_sb[:, :], in_=maskv[:, :])
        nc.gpsimd.dma_start(out=initm_sb[:, :], in_=initm[:, :])
        nc.vector.memset(alpha[:, :], 0.0)

        def preprocess(k, i):
            """Stream exp-tile k (ORDER position i); returns its DRAM scratch."""
            pt = pred_pool.tile([128, C], F8, tag="pt")
            if i < NSPLIT:
                # split across BOTH queues in parallel for a fast ramp-up
                for h, q in enumerate((nc.sync, nc.scalar)):
                    rows = slice(64 * h, 64 * h + 64)
                    boff = 4 * h * T * C
                    if k < NPH:
                        q.dma_start(
                            out=pt[rows, :],
                            in_=pred[4 * h:4 * h + 4,
                                     k * TBLK:(k + 1) * TBLK, :],
                            max_dma_last_dim=3328)
                    else:
                        q.dma_start(
                            out=pt[rows, :],
                            in_=AP(pred, boff + (k * TBLK + TBLK - 1) * C,
                                   [[T * C, 4], [-C, TBLK], [1, C]]),
                            max_dma_last_dim=3328)
            else:
                q = nc.sync if i % 2 == 0 else nc.scalar
                if k < NPH:
                    q.dma_start(out=pt[:, :],
                                in_=pred[:, k * TBLK:(k + 1) * TBLK, :])
                else:
                    # time-reversed load: partition r = b*16 + (15 - t_local)
                    q.dma_start(out=pt[:, :],
                                in_=AP(pred, (k * TBLK + TBLK - 1) * C,
                                       [[T * C, BPC], [-C, TBLK], [1, C]]))
            et = exp_pool.tile([128, CP], FP, tag="et")
            nc.vector.memset(et[:, C:CP], 0.0)
            nc.scalar.activation(
                out=et[:, 0:C], in_=pt[:, :],
                func=mybir.ActivationFunctionType.Exp,
                accum_out=sacc[:, i:i + 1],
            )
            g = small.tile([128, GC], FP, tag="g")
            nc.gpsimd.ap_gather(
                g[:, :], et[:, 0:CP],
                idxf_sb[:, :] if k < NPH else idxb_sb[:, :],
                channels=128, num_elems=CP, d=1, num_idxs=GC,
            )
            # cast f32 -> bf16 on the SWDGE write leg
            pscr = dram_pool.tile([BPC, TBLK, GC], BF, name=f"pscr{i}")
            nc.gpsimd.dma_start(out=pscr[:, :, :], in_=g[:, :])
            return pscr

        PING, PONG = 0, 64
        jn = 0
        for m in range(NPH):
            pscr_f = preprocess(m, 2 * m)
            pscr_b = preprocess(NBLK - 1 - m, 2 * m + 1)
            pdp = pdp_pool.tile([ROWS, TBLK, GC], BF, tag="pdp")
            nc.sync.dma_start(out=pdp[0:BPC, :, :], in_=pscr_f[:, :, :])
            nc.scalar.dma_start(out=pdp[BPC:2 * BPC, :, :], in_=pscr_b[:, :, :])
            for ti in range(TBLK):
                j = m * TBLK + ti
                if j == 0:
                    nc.vector.tensor_mul(
                        alpha[:, PING + 2:PING + 2 + S],
                        pdp[:, 0, 0:S], initm_sb[:, 0:S],
                    )
                    continue
                src = PING if j % 2 == 1 else PONG
                dst = PONG if j % 2 == 1 else PING
                vt = small.tile([ROWS, S], BF, tag="vt")
                nc.vector.tensor_mul(
                    vt[:, :], alpha[:, src:src + S], maskv_sb[:, 0:S]
                )
                ut = small.tile([ROWS, S], BF, tag="ut")
                nc.vector.tensor_add(
                    ut[:, :], alpha[:, src + 2:src + 2 + S],
                    alpha[:, src + 1:src + 1 + S],
                )
                nc.vector.tensor_add(ut[:, :], ut[:, :], vt[:, :])
                adst = alpha[:, dst + 2:dst + 2 + S]
                nc.vector.tensor_mul(adst, ut[:, :], pdp[:, ti, 0:S])
                if j % 8 == 7:
                    nc.vector.tensor_reduce(
                        out=cbuf[:, jn:jn + 1], in_=adst,
                        axis=mybir.AxisListType.X, op=ADD,
                    )
                    nc.vector.reciprocal(rcn[:, :], cbuf[:, jn:jn + 1])
                    nc.vector.tensor_scalar_mul(adst, adst, rcn[:, 0:1])
                    jn += 1

        assert jn == NNORM
        # final state after j=79 (odd) lives in the PONG half
        nc.sync.dma_start(out=out_alpha[:, :], in_=alpha[:, PONG:PONG + GC])
        nc.sync.dma_start(out=out_c[:, :], in_=cbuf[:, :])
        nc.sync.dma_start(out=out_s[:, :], in_=sacc[:, :])
    nc.compile()
    return nc


_CACHE: dict = {}


def _get_nc() -> bass.Bass:
    if "nc" not in _CACHE:
        _CACHE["nc"] = build_nc()
    return _CACHE["nc"]


LAST_RESULTS = None


def kernel(pred, targets, targets_lengths) -> np.ndarray:
    global LAST_RESULTS
    pred = np.ascontiguousarray(np.asarray(pred, dtype=np.float32))
    targets = np.asarray(targets).astype(np.int64)
    tl = np.asarray(targets_lengths).astype(np.int64)
    assert pred.shape == (B, T, C), pred.shape
    assert targets.shape == (B, L)

    # host prep: extended labels, skip masks, gather index tables
    ext = np.zeros((B, S), dtype=np.int64)
    ext[:, 1::2] = targets
    skip = np.zeros((B, S), dtype=np.float32)
    skip[:, 2:] = ((ext[:, 2:] != 0) & (ext[:, 2:] != ext[:, :-2])).astype(np.float32)

    pred8 = pred.astype(ml_dtypes.float8_e4m3fn)

    in_maps = []
    for c in range(N_CORES):
        lo = c * BPC
        idxf_h = np.full((128, GC // 16), ZCOL, dtype=np.int16)
        idxb_h = np.full((128, GC // 16), ZCOL, dtype=np.int16)
        mv = np.zeros((ROWS, GC), dtype=np.float32)
        im = np.zeros((ROWS, GC), dtype=np.float32)
        for g in range(BPC):
            b = lo + g
            band = 2 * int(tl[b])           # reachable states: 0..band
            for j in range(S):
                if j <= band:
                    idxf_h[16 * g + (j % 16), j // 16] = ext[b, j]
                sm = S - 1 - j              # mirrored state for the bwd DP
                if sm <= band:
                    idxb_h[16 * g + (j % 16), j // 16] = ext[b, sm]
            mv[g, :S] = skip[b]
            # mirrored skip for beta: m~[s~] = skip[52 - s~] for s~ in [2, 50]
            for st in range(2, S):
                mv[BPC + g, st] = skip[b, 52 - st]
            im[g, 0:2] = 1.0
            im[BPC + g, (S - 1) - band] = 1.0
            im[BPC + g, S - band] = 1.0
        in_maps.append({
            "pred": np.ascontiguousarray(pred8[lo:lo + BPC]),
            "idxf": idxf_h,
            "idxb": idxb_h,
            "maskv": mv.astype(ml_dtypes.bfloat16),
            "initm": im.astype(ml_dtypes.bfloat16),
        })

    nc = _get_nc()
    LAST_RESULTS = run_bass_kernel_spmd(nc, in_maps, core_ids=list(range(N_CORES)))
    results = LAST_RESULTS.results

    # host epilogue (f64, 64 samples)
    per_sample = np.zeros(B, dtype=np.float64)
    for c in range(N_CORES):
        oa = np.asarray(results[c]["out_alpha"]).astype(np.float64)   # [16, 64]
        oc = results[c]["out_c"].astype(np.float64)                   # [16, 10]
        os_ = results[c]["out_s"].astype(np.float64)                  # [128, 10]
        for g in range(BPC):
            b = c * BPC + g
            a = oa[g, 2:2 + S]                    # alpha_79[s]
            beta = oa[BPC + g, 2:2 + S][::-1]     # beta_80[s] (un-mirrored)
            gam = beta.copy()
            gam[:-1] += beta[1:]
            gam[:-2] += skip[b, 2:].astype(np.float64) * beta[2:]
            dot = float(np.dot(a, gam))
            cf = oc[g]
            cb = oc[BPC + g]
            sv = os_[16 * g:16 * (g + 1), :].reshape(-1)   # all 160 denominators
            if (dot <= 0.0 or not np.isfinite(dot)
                    or np.any(cf <= 0.0) or np.any(cb <= 0.0)
                    or np.any(sv <= 0.0) or not np.all(np.isfinite(cf))
                    or not np.all(np.isfinite(cb))):
                raw = np.inf
            else:
                raw = -(np.log(dot) + np.log(cf).sum() + np.log(cb).sum()
                        - np.log(sv).sum())
            safe = 0.0 if np.isinf(raw) else raw
            per_sample[b] = safe / max(int(tl[b]), 1)
    return np.asarray(per_sample.mean(), dtype=np.float32)


# revision 16
# speedup vs baseline: 1.0068x; 1.0068x over previous
"""CTC loss kernel for Trainium2 (8 NeuronCores, batch-parallel).

Strategy (v3)
-------------
Batch B=64 sharded 8 samples/core. pred is cast to fp8(e4m3) on the HOST, so
the device streams 8.5 MB/core instead of 34 MB (HBM read is the roofline:
measured ~125-170 GB/s/core regardless of queue mix, so fewer bytes win).
Per core, ten [128, 6625] tiles (partition p = b*16 + r) stream through:

  1. DMA in, alternating the two HWDGE queues (sync/scalar); the first two
     tile PAIRS are split into sample-halves issued on BOTH queues in
     parallel so the DP pipeline starts ~35us earlier. Tiles 0-4 load
     time-ascending; tiles 9-5 load TIME-REVERSED via a negative-stride
     DRAM access pattern (r = 15 - t_local).
  2. ScalarE: Exp fp8->f32 with fused per-row accumulate. The softmax
     denominators s are NOT applied on device - they are written out and
     folded back on the host (log-domain, f64). A 31-column zero pad is
     memset next to the exp output.
  3. GPSIMD ap_gather picks 64 columns per row: the extended-label class
     columns, with per-sample band masking (state > 2*target_len) baked in
     by pointing masked states at the zero pad column. Tiles 0-4 gather
     forward states; tiles 9-5 gather MIRRORED states for the backward DP.
  4. Regroup (b*16+r, s) -> (b, r, s) via DRAM scratch (cast to bf16 on the
     way out) into a [16, 16, 64] bf16 tile: rows 0-7 = forward p, rows
     8-15 = backward p-tilde.

The CTC recursion is split in half: alpha runs forward over t=0..79 (rows
0-7) while beta runs backward over t=159..80 (rows 8-15, state-mirrored so
its shifts match alpha's). Both advance in the SAME VectorE instructions
([16, 51] bf16 tiles), halving the serial chain to 80 steps:

  new[s] = (cur[s] + cur[s-1] + skip[s]*cur[s-2]) * p[t, s]

Renormalization every 8 steps is POSTPONED: the sum is computed on GPSIMD
(off the VectorE critical path) and applied as a single tensor-scalar
divide fused after the next step's multiply; the final step renormalizes
in-chain. Host epilogue: L = sum_s alpha_79[s] * (beta_80[s] + beta_80[s+1]
+ skip[s+2]*beta_80[s+2]), NLL = -(log L + sum log c - sum_t log s_t),
zero-infinity, length division, batch mean (f64, 64 samples).
"""

from contextlib import ExitStack

import numpy as np
import ml_dtypes

import concourse.bass as bass
import concourse.tile as tile
from concourse import bacc, mybir
from concourse.ap import AP
from concourse.bass_utils import run_bass_kernel_spmd

N_CORES = 8
B = 64
T = 160
C = 6625
L = 25
S = 2 * L + 1           # 51 extended states
BPC = B // N_CORES      # 8 samples per core
TBLK = 16               # time steps per streamed tile
NBLK = T // TBLK        # 10 tiles per core
GC = 64                 # gather columns (51 states padded to 64)
CP = C + 31             # exp tile free size incl. zero pad columns
ZCOL = C                # index of the (memset) zero column
ROWS = 16               # 0-7 fwd alpha, 8-15 bwd beta (mirrored)
NPH = 5                 # DP phases (80 steps / 16)
NNORM = 10              # renorm sums at j = 7, 15, ..., 79
NSPLIT = 2              # ORDER positions whose DMA is split across both queues
ORDER = [0, 9, 1, 8, 2, 7, 3, 6, 4, 5]

FP = mybir.dt.float32
BF = mybir.dt.bfloat16
F8 = mybir.dt.float8e4
ADD = mybir.AluOpType.add
DIV = mybir.AluOpType.divide


def build_nc() -> bass.Bass:
    nc = bacc.Bacc("TRN2", target_bir_lowering=False, debug=False,
                   num_devices=N_CORES)
    pred = nc.dram_tensor("pred", [BPC, T, C], F8, kind="ExternalInput")
    idxf = nc.dram_tensor("idxf", [128, GC // 16], mybir.dt.int16,
                          kind="ExternalInput")
    idxb = nc.dram_tensor("idxb", [128, GC // 16], mybir.dt.int16,
                          kind="ExternalInput")
    maskv = nc.dram_tensor("maskv", [ROWS, GC], BF, kind="ExternalInput")
    initm = nc.dram_tensor("initm", [ROWS, GC], BF, kind="ExternalInput")
    out_alpha = nc.dram_tensor("out_alpha", [ROWS, GC], BF, kind="ExternalOutput")
    out_c = nc.dram_tensor("out_c", [ROWS, NNORM], FP, kind="ExternalOutput")
    out_s = nc.dram_tensor("out_s", [128, NBLK], FP, kind="ExternalOutput")

    with tile.TileContext(nc) as tc, ExitStack() as ctx:
        pred_pool = ctx.enter_context(tc.tile_pool(name="pred_pool", bufs=3))
        exp_pool = ctx.enter_context(tc.tile_pool(name="exp_pool", bufs=3))
        small = ctx.enter_context(tc.tile_pool(name="small", bufs=3))
        pdp_pool = ctx.enter_context(tc.tile_pool(name="pdp_pool", bufs=3))
        dram_pool = ctx.enter_context(
            tc.tile_pool(name="pscr_pool", bufs=4, space="DRAM"))

        def single(shape, dtype, name):
            t, free = tc.tile(shape, dtype, name=name)
            ctx.callback(free)
            return t

        idxf_sb = single([128, GC // 16], mybir.dt.int16, "idxf_sb")
        idxb_sb = single([128, GC // 16], mybir.dt.int16, "idxb_sb")
        maskv_sb = single([ROWS, GC], BF, "maskv_sb")
        initm_sb = single([ROWS, GC], BF, "initm_sb")
        # ping/pong alpha with 2 guard columns each: ping states at 2..52,
        # pong states at 66..116; guards stay zero forever.
        alpha = single([ROWS, 128], BF, "alpha")
        cbuf = single([ROWS, NNORM], FP, "cbuf")
        rcn = single([ROWS, 1], FP, "rcn")
        sacc = single([128, NBLK], FP, "sacc")

        # constants go on the gpsimd queue so the first pred DMA leads sync
        nc.gpsimd.dma_start(out=idxf_sb[:, :], in_=idxf[:, :])
        nc.gpsimd.dma_start(out=idxb_sb[:, :], in_=idxb[:, :])
        nc.gpsimd.dma_start(out=maskv_sb[:, :], in_=maskv[:, :])
        nc.gpsimd.dma_start(out=initm_sb[:, :], in_=initm[:, :])
        nc.vector.memset(alpha[:, :], 0.0)

        def preprocess(k, i):
            """Stream exp-tile k (ORDER position i); returns its DRAM scratch."""
            pt = pred_pool.tile([128, C], F8, tag="pt")
            if i < NSPLIT:
                # split across BOTH queues in parallel for a fast ramp-up
                for h, q in enumerate((nc.sync, nc.scalar)):
                    rows = slice(64 * h, 64 * h + 64)
                    boff = 4 * h * T * C
                    if k < NPH:
                        q.dma_start(
                            out=pt[rows, :],
                            in_=pred[4 * h:4 * h + 4,
                                     k * TBLK:(k + 1) * TBLK, :],
                            max_dma_last_dim=3328)
                    else:
                        q.dma_start(
                            out=pt[rows, :],
                            in_=AP(pred, boff + (k * TBLK + TBLK - 1) * C,
                                   [[T * C, 4], [-C, TBLK], [1, C]]),
                            max_dma_last_dim=3328)
            else:
                q = nc.sync if i % 2 == 0 else nc.scalar
                if k < NPH:
                    q.dma_start(out=pt[:, :],
                                in_=pred[:, k * TBLK:(k + 1) * TBLK, :])
                else:
                    # time-reversed load: partition r = b*16 + (15 - t_local)
                    q.dma_start(out=pt[:, :],
                                in_=AP(pred, (k * TBLK + TBLK - 1) * C,
                                       [[T * C, BPC], [-C, TBLK], [1, C]]))
            et = exp_pool.tile([128, CP], FP, tag="et")
            nc.vector.memset(et[:, C:CP], 0.0)
            nc.scalar.activation(
                out=et[:, 0:C], in_=pt[:, :],
                func=mybir.ActivationFunctionType.Exp,
                accum_out=sacc[:, i:i + 1],
            )
            g = small.tile([128, GC], FP, tag="g")
            nc.gpsimd.ap_gather(
                g[:, :], et[:, 0:CP],
                idxf_sb[:, :] if k < NPH else idxb_sb[:, :],
                channels=128, num_elems=CP, d=1, num_idxs=GC,
            )
            # cast f32 -> bf16 on the SWDGE write leg
            pscr = dram_pool.tile([BPC, TBLK, GC], BF, name=f"pscr{i}")
            nc.gpsimd.dma_start(out=pscr[:, :, :], in_=g[:, :])
            return pscr

        PING, PONG = 0, 64
        jn = 0
        for m in range(NPH):
            pscr_f = preprocess(m, 2 * m)
            pscr_b = preprocess(NBLK - 1 - m, 2 * m + 1)
            pdp = pdp_pool.tile([ROWS, TBLK, GC], BF, tag="pdp")
            nc.sync.dma_start(out=pdp[0:BPC, :, :], in_=pscr_f[:, :, :])
            nc.scalar.dma_start(out=pdp[BPC:2 * BPC, :, :], in_=pscr_b[:, :, :])
            for ti in range(TBLK):
                j = m * TBLK + ti
                if j == 0:
                    nc.vector.tensor_mul(
                        alpha[:, PING + 2:PING + 2 + S],
                        pdp[:, 0, 0:S], initm_sb[:, 0:S],
                    )
                    continue
                src = PING if j % 2 == 1 else PONG
                dst = PONG if j % 2 == 1 else PING
                vt = small.tile([ROWS, S], BF, tag="vt")
                nc.vector.tensor_mul(
                    vt[:, :], alpha[:, src:src + S], maskv_sb[:, 0:S]
                )
                ut = small.tile([ROWS, S], BF, tag="ut")
                nc.vector.tensor_add(
                    ut[:, :], alpha[:, src + 2:src + 2 + S],
                    alpha[:, src + 1:src + 1 + S],
                )
                nc.vector.tensor_add(ut[:, :], ut[:, :], vt[:, :])
                adst = alpha[:, dst + 2:dst + 2 + S]
                nc.vector.tensor_mul(adst, ut[:, :], pdp[:, ti, 0:S])
                if j % 8 == 7:
                    nc.vector.tensor_reduce(
                        out=cbuf[:, jn:jn + 1], in_=adst,
                        axis=mybir.AxisListType.X, op=ADD,
                    )
                    nc.vector.reciprocal(rcn[:, :], cbuf[:, jn:jn + 1])
                    nc.vector.tensor_scalar_mul(adst, adst, rcn[:, 0:1])
                    jn += 1

        assert jn == NNORM
        # final state after j=79 (odd) lives in the PONG half
        nc.sync.dma_start(out=out_alpha[:, :], in_=alpha[:, PONG:PONG + GC])
        nc.sync.dma_start(out=out_c[:, :], in_=cbuf[:, :])
        nc.sync.dma_start(out=out_s[:, :], in_=sacc[:, :])
    nc.compile()
    return nc


_CACHE: dict = {}


def _get_nc() -> bass.Bass:
    if "nc" not in _CACHE:
        _CACHE["nc"] = build_nc()
    return _CACHE["nc"]


LAST_RESULTS = None


def kernel(pred, targets, targets_lengths) -> np.ndarray:
    global LAST_RESULTS
    pred = np.ascontiguousarray(np.asarray(pred, dtype=np.float32))
    targets = np.asarray(targets).astype(np.int64)
    tl = np.asarray(targets_lengths).astype(np.int64)
    assert pred.shape == (B, T, C), pred.shape
    assert targets.shape == (B, L)

    # host prep: extended labels, skip masks, gather index tables
    ext = np.zeros((B, S), dtype=np.int64)
    ext[:, 1::2] = targets
    skip = np.zeros((B, S), dtype=np.float32)
    skip[:, 2:] = ((ext[:, 2:] != 0) & (ext[:, 2:] != ext[:, :-2])).astype(np.float32)

    pred8 = pred.astype(ml_dtypes.float8_e4m3fn)

    in_maps = []
    for c in range(N_CORES):
        lo = c * BPC
        idxf_h = np.full((128, GC // 16), ZCOL, dtype=np.int16)
        idxb_h = np.full((128, GC // 16), ZCOL, dtype=np.int16)
        mv = np.zeros((ROWS, GC), dtype=np.float32)
        im = np.zeros((ROWS, GC), dtype=np.float32)
        for g in range(BPC):
            b = lo + g
            band = 2 * int(tl[b])           # reachable states: 0..band
            for j in range(S):
                if j <= band:
                    idxf_h[16 * g + (j % 16), j // 16] = ext[b, j]
                sm = S - 1 - j              # mirrored state for the bwd DP
                if sm <= band:
                    idxb_h[16 * g + (j % 16), j // 16] = ext[b, sm]
            mv[g, :S] = skip[b]
            # mirrored skip for beta: m~[s~] = skip[52 - s~] for s~ in [2, 50]
            for st in range(2, S):
                mv[BPC + g, st] = skip[b, 52 - st]
            im[g, 0:2] = 1.0
            im[BPC + g, (S - 1) - band] = 1.0
            im[BPC + g, S - band] = 1.0
        in_maps.append({
            "pred": np.ascontiguousarray(pred8[lo:lo + BPC]),
            "idxf": idxf_h,
            "idxb": idxb_h,
            "maskv": mv.astype(ml_dtypes.bfloat16),
            "initm": im.astype(ml_dtypes.bfloat16),
        })

    nc = _get_nc()
    LAST_RESULTS = run_bass_kernel_spmd(nc, in_maps, core_ids=list(range(N_CORES)))
    results = LAST_RESULTS.results

    # host epilogue (f64, 64 samples)
    per_sample = np.zeros(B, dtype=np.float64)
    for c in range(N_CORES):
        oa = np.asarray(results[c]["out_alpha"]).astype(np.float64)   # [16, 64]
        oc = results[c]["out_c"].astype(np.float64)                   # [16, 10]
        os_ = results[c]["out_s"].astype(np.float64)                  # [128, 10]
        for g in range(BPC):
            b = c * BPC + g
            a = oa[g, 2:2 + S]                    # alpha_79[s]
            beta = oa[BPC + g, 2:2 + S][::-1]     # beta_80[s] (un-mirrored)
            gam = beta.copy()
            gam[:-1] += beta[1:]
            gam[:-2] += skip[b, 2:].astype(np.float64) * beta[2:]
            dot = float(np.dot(a, gam))
            cf = oc[g]
            cb = oc[BPC + g]
            sv = os_[16 * g:16 * (g + 1), :].reshape(-1)   # all 160 denominators
            if (dot <= 0.0 or not np.isfinite(dot)
                    or np.any(cf <= 0.0) or np.any(cb <= 0.0)
                    or np.any(sv <= 0.0) or not np.all(np.isfinite(cf))
                    or not np.all(np.isfinite(cb))):
                raw = np.inf
            else:
                raw = -(np.log(dot) + np.log(cf).sum() + np.log(cb).sum()
                        - np.log(sv).sum())
            safe = 0.0 if np.isinf(raw) else raw
            per_sample[b] = safe / max(int(tl[b]), 1)
    return np.asarray(per_sample.mean(), dtype=np.float32)


# revision 17
# speedup vs baseline: 1.0816x; 1.0744x over previous
"""CTC loss kernel for Trainium2 (8 NeuronCores, batch-parallel).

Strategy (v3)
-------------
Batch B=64 sharded 8 samples/core. pred is cast to fp8(e4m3) on the HOST, so
the device streams 8.5 MB/core instead of 34 MB (HBM read is the roofline:
measured ~125-170 GB/s/core regardless of queue mix, so fewer bytes win).
Per core, ten [128, 6625] tiles (partition p = b*16 + r) stream through:

  1. DMA in, alternating the two HWDGE queues (sync/scalar); the first two
     tile PAIRS are split into sample-halves issued on BOTH queues in
     parallel so the DP pipeline starts ~35us earlier. Tiles 0-4 load
     time-ascending; tiles 9-5 load TIME-REVERSED via a negative-stride
     DRAM access pattern (r = 15 - t_local).
  2. ScalarE: Exp fp8->f32 with fused per-row accumulate. The softmax
     denominators s are NOT applied on device - they are written out and
     folded back on the host (log-domain, f64). A 31-column zero pad is
     memset next to the exp output.
  3. GPSIMD ap_gather picks 64 columns per row: the extended-label class
     columns, with per-sample band masking (state > 2*target_len) baked in
     by pointing masked states at the zero pad column. Tiles 0-4 gather
     forward states; tiles 9-5 gather MIRRORED states for the backward DP.
  4. Regroup (b*16+r, s) -> (b, r, s) via DRAM scratch (cast to bf16 on the
     way out) into a [16, 16, 64] bf16 tile: rows 0-7 = forward p, rows
     8-15 = backward p-tilde.

The CTC recursion is split in half: alpha runs forward over t=0..79 (rows
0-7) while beta runs backward over t=159..80 (rows 8-15, state-mirrored so
its shifts match alpha's). Both advance in the SAME VectorE instructions
([16, 51] bf16 tiles), halving the serial chain to 80 steps:

  new[s] = (cur[s] + cur[s-1] + skip[s]*cur[s-2]) * p[t, s]

Renormalization every 8 steps is POSTPONED: the sum is computed on GPSIMD
(off the VectorE critical path) and applied as a single tensor-scalar
divide fused after the next step's multiply; the final step renormalizes
in-chain. Host epilogue: L = sum_s alpha_79[s] * (beta_80[s] + beta_80[s+1]
+ skip[s+2]*beta_80[s+2]), NLL = -(log L + sum log c - sum_t log s_t),
zero-infinity, length division, batch mean (f64, 64 samples).
"""

from contextlib import ExitStack

import numpy as np
import ml_dtypes

import concourse.bass as bass
import concourse.tile as tile
from concourse import bacc, mybir
from concourse.ap import AP
from concourse.bass_utils import run_bass_kernel_spmd

N_CORES = 8
B = 64
T = 160
C = 6625
L = 25
S = 2 * L + 1           # 51 extended states
BPC = B // N_CORES      # 8 samples per core
TBLK = 16               # time steps per streamed tile
NBLK = T // TBLK        # 10 tiles per core
GC = 64                 # gather columns (51 states padded to 64)
CP = C + 31             # exp tile free size incl. zero pad columns
ZCOL = C                # index of the (memset) zero column
ROWS = 16               # 0-7 fwd alpha, 8-15 bwd beta (mirrored)
NPH = 5                 # DP phases (80 steps / 16)
NNORM = 10              # renorm sums at j = 7, 15, ..., 79
NSPLIT = 0              # ORDER positions whose DMA is split across both queues
ORDER = [0, 9, 1, 8, 2, 7, 3, 6, 4, 5]

FP = mybir.dt.float32
BF = mybir.dt.bfloat16
F8 = mybir.dt.float8e4
ADD = mybir.AluOpType.add
DIV = mybir.AluOpType.divide


def build_nc() -> bass.Bass:
    nc = bacc.Bacc("TRN2", target_bir_lowering=False, debug=False,
                   num_devices=N_CORES)
    pred = nc.dram_tensor("pred", [BPC, T, C], F8, kind="ExternalInput")
    idxf = nc.dram_tensor("idxf", [128, GC // 16], mybir.dt.int16,
                          kind="ExternalInput")
    idxb = nc.dram_tensor("idxb", [128, GC // 16], mybir.dt.int16,
                          kind="ExternalInput")
    maskv = nc.dram_tensor("maskv", [ROWS, GC], BF, kind="ExternalInput")
    initm = nc.dram_tensor("initm", [ROWS, GC], BF, kind="ExternalInput")
    out_alpha = nc.dram_tensor("out_alpha", [ROWS, GC], BF, kind="ExternalOutput")
    out_c = nc.dram_tensor("out_c", [ROWS, NNORM], FP, kind="ExternalOutput")
    out_s = nc.dram_tensor("out_s", [128, NBLK], FP, kind="ExternalOutput")

    with tile.TileContext(nc) as tc, ExitStack() as ctx:
        pred_pool = ctx.enter_context(tc.tile_pool(name="pred_pool", bufs=3))
        exp_pool = ctx.enter_context(tc.tile_pool(name="exp_pool", bufs=3))
        small = ctx.enter_context(tc.tile_pool(name="small", bufs=3))
        pdp_pool = ctx.enter_context(tc.tile_pool(name="pdp_pool", bufs=3))
        dram_pool = ctx.enter_context(
            tc.tile_pool(name="pscr_pool", bufs=4, space="DRAM"))

        def single(shape, dtype, name):
            t, free = tc.tile(shape, dtype, name=name)
            ctx.callback(free)
            return t

        idxf_sb = single([128, GC // 16], mybir.dt.int16, "idxf_sb")
        idxb_sb = single([128, GC // 16], mybir.dt.int16, "idxb_sb")
        maskv_sb = single([ROWS, GC], BF, "maskv_sb")
        initm_sb = single([ROWS, GC], BF, "initm_sb")
        # ping/pong alpha with 2 guard columns each: ping states at 2..52,
        # pong states at 66..116; guards stay zero forever.
        alpha = single([ROWS, 128], BF, "alpha")
        cbuf = single([ROWS, NNORM], FP, "cbuf")
        rcn = single([ROWS, 1], FP, "rcn")
        sacc = single([128, NBLK], FP, "sacc")

        # constants go on the gpsimd queue so the first pred DMA leads sync
        nc.gpsimd.dma_start(out=idxf_sb[:, :], in_=idxf[:, :])
        nc.gpsimd.dma_start(out=idxb_sb[:, :], in_=idxb[:, :])
        nc.gpsimd.dma_start(out=maskv_sb[:, :], in_=maskv[:, :])
        nc.gpsimd.dma_start(out=initm_sb[:, :], in_=initm[:, :])
        nc.vector.memset(alpha[:, :], 0.0)

        def preprocess(k, i):
            """Stream exp-tile k (ORDER position i); returns its DRAM scratch."""
            pt = pred_pool.tile([128, C], F8, tag="pt")
            if i < NSPLIT:
                # split across BOTH queues in parallel for a fast ramp-up
                for h, q in enumerate((nc.sync, nc.scalar)):
                    rows = slice(64 * h, 64 * h + 64)
                    boff = 4 * h * T * C
                    if k < NPH:
                        q.dma_start(
                            out=pt[rows, :],
                            in_=pred[4 * h:4 * h + 4,
                                     k * TBLK:(k + 1) * TBLK, :],
                            max_dma_last_dim=3328)
                    else:
                        q.dma_start(
                            out=pt[rows, :],
                            in_=AP(pred, boff + (k * TBLK + TBLK - 1) * C,
                                   [[T * C, 4], [-C, TBLK], [1, C]]),
                            max_dma_last_dim=3328)
            else:
                q = nc.sync if i % 2 == 0 else nc.scalar
                if k < NPH:
                    q.dma_start(out=pt[:, :],
                                in_=pred[:, k * TBLK:(k + 1) * TBLK, :])
                else:
                    # time-reversed load: partition r = b*16 + (15 - t_local)
                    q.dma_start(out=pt[:, :],
                                in_=AP(pred, (k * TBLK + TBLK - 1) * C,
                                       [[T * C, BPC], [-C, TBLK], [1, C]]))
            et = exp_pool.tile([128, CP], FP, tag="et")
            nc.vector.memset(et[:, C:CP], 0.0)
            nc.scalar.activation(
                out=et[:, 0:C], in_=pt[:, :],
                func=mybir.ActivationFunctionType.Exp,
                accum_out=sacc[:, i:i + 1],
            )
            g = small.tile([128, GC], FP, tag="g")
            nc.gpsimd.ap_gather(
                g[:, :], et[:, 0:CP],
                idxf_sb[:, :] if k < NPH else idxb_sb[:, :],
                channels=128, num_elems=CP, d=1, num_idxs=GC,
            )
            # cast f32 -> bf16 on the SWDGE write leg
            pscr = dram_pool.tile([BPC, TBLK, GC], BF, name=f"pscr{i}")
            nc.gpsimd.dma_start(out=pscr[:, :, :], in_=g[:, :])
            return pscr

        PING, PONG = 0, 64
        jn = 0
        for m in range(NPH):
            pscr_f = preprocess(m, 2 * m)
            pscr_b = preprocess(NBLK - 1 - m, 2 * m + 1)
            pdp = pdp_pool.tile([ROWS, TBLK, GC], BF, tag="pdp")
            nc.sync.dma_start(out=pdp[0:BPC, :, :], in_=pscr_f[:, :, :])
            nc.scalar.dma_start(out=pdp[BPC:2 * BPC, :, :], in_=pscr_b[:, :, :])
            for ti in range(TBLK):
                j = m * TBLK + ti
                if j == 0:
                    nc.vector.tensor_mul(
                        alpha[:, PING + 2:PING + 2 + S],
                        pdp[:, 0, 0:S], initm_sb[:, 0:S],
                    )
                    continue
                src = PING if j % 2 == 1 else PONG
                dst = PONG if j % 2 == 1 else PING
                vt = small.tile([ROWS, S], BF, tag="vt")
                nc.vector.tensor_mul(
                    vt[:, :], alpha[:, src:src + S], maskv_sb[:, 0:S]
                )
                ut = small.tile([ROWS, S], BF, tag="ut")
                nc.vector.tensor_add(
                    ut[:, :], alpha[:, src + 2:src + 2 + S],
                    alpha[:, src + 1:src + 1 + S],
                )
                nc.vector.tensor_add(ut[:, :], ut[:, :], vt[:, :])
                adst = alpha[:, dst + 2:dst + 2 + S]
                nc.vector.tensor_mul(adst, ut[:, :], pdp[:, ti, 0:S])
                if j % 8 == 7:
                    nc.vector.tensor_reduce(
                        out=cbuf[:, jn:jn + 1], in_=adst,
                        axis=mybir.AxisListType.X, op=ADD,
                    )
                    nc.vector.reciprocal(rcn[:, :], cbuf[:, jn:jn + 1])
                    nc.vector.tensor_scalar_mul(adst, adst, rcn[:, 0:1])
                    jn += 1

        assert jn == NNORM
        # final state after j=79 (odd) lives in the PONG half
        nc.sync.dma_start(out=out_alpha[:, :], in_=alpha[:, PONG:PONG + GC])
        nc.sync.dma_start(out=out_c[:, :], in_=cbuf[:, :])
        nc.sync.dma_start(out=out_s[:, :], in_=sacc[:, :])
    nc.compile()
    return nc


_CACHE: dict = {}


def _get_nc() -> bass.Bass:
    if "nc" not in _CACHE:
        _CACHE["nc"] = build_nc()
    return _CACHE["nc"]


LAST_RESULTS = None


def kernel(pred, targets, targets_lengths) -> np.ndarray:
    global LAST_RESULTS
    pred = np.ascontiguousarray(np.asarray(pred, dtype=np.float32))
    targets = np.asarray(targets).astype(np.int64)
    tl = np.asarray(targets_lengths).astype(np.int64)
    assert pred.shape == (B, T, C), pred.shape
    assert targets.shape == (B, L)

    # host prep: extended labels, skip masks, gather index tables
    ext = np.zeros((B, S), dtype=np.int64)
    ext[:, 1::2] = targets
    skip = np.zeros((B, S), dtype=np.float32)
    skip[:, 2:] = ((ext[:, 2:] != 0) & (ext[:, 2:] != ext[:, :-2])).astype(np.float32)

    pred8 = pred.astype(ml_dtypes.float8_e4m3fn)

    in_maps = []
    for c in range(N_CORES):
        lo = c * BPC
        idxf_h = np.full((128, GC // 16), ZCOL, dtype=np.int16)
        idxb_h = np.full((128, GC // 16), ZCOL, dtype=np.int16)
        mv = np.zeros((ROWS, GC), dtype=np.float32)
        im = np.zeros((ROWS, GC), dtype=np.float32)
        for g in range(BPC):
            b = lo + g
            band = 2 * int(tl[b])           # reachable states: 0..band
            for j in range(S):
                if j <= band:
                    idxf_h[16 * g + (j % 16), j // 16] = ext[b, j]
                sm = S - 1 - j              # mirrored state for the bwd DP
                if sm <= band:
                    idxb_h[16 * g + (j % 16), j // 16] = ext[b, sm]
            mv[g, :S] = skip[b]
            # mirrored skip for beta: m~[s~] = skip[52 - s~] for s~ in [2, 50]
            for st in range(2, S):
                mv[BPC + g, st] = skip[b, 52 - st]
            im[g, 0:2] = 1.0
            im[BPC + g, (S - 1) - band] = 1.0
            im[BPC + g, S - band] = 1.0
        in_maps.append({
            "pred": np.ascontiguousarray(pred8[lo:lo + BPC]),
            "idxf": idxf_h,
            "idxb": idxb_h,
            "maskv": mv.astype(ml_dtypes.bfloat16),
            "initm": im.astype(ml_dtypes.bfloat16),
        })

    nc = _get_nc()
    LAST_RESULTS = run_bass_kernel_spmd(nc, in_maps, core_ids=list(range(N_CORES)))
    results = LAST_RESULTS.results

    # host epilogue (f64, 64 samples)
    per_sample = np.zeros(B, dtype=np.float64)
    for c in range(N_CORES):
        oa = np.asarray(results[c]["out_alpha"]).astype(np.float64)   # [16, 64]
        oc = results[c]["out_c"].astype(np.float64)                   # [16, 10]
        os_ = results[c]["out_s"].astype(np.float64)                  # [128, 10]
        for g in range(BPC):
            b = c * BPC + g
            a = oa[g, 2:2 + S]                    # alpha_79[s]
            beta = oa[BPC + g, 2:2 + S][::-1]     # beta_80[s] (un-mirrored)
            gam = beta.copy()
            gam[:-1] += beta[1:]
            gam[:-2] += skip[b, 2:].astype(np.float64) * beta[2:]
            dot = float(np.dot(a, gam))
            cf = oc[g]
            cb = oc[BPC + g]
            sv = os_[16 * g:16 * (g + 1), :].reshape(-1)   # all 160 denominators
            if (dot <= 0.0 or not np.isfinite(dot)
                    or np.any(cf <= 0.0) or np.any(cb <= 0.0)
                    or np.any(sv <= 0.0) or not np.all(np.isfinite(cf))
                    or not np.all(np.isfinite(cb))):
                raw = np.inf
            else:
                raw = -(np.log(dot) + np.log(cf).sum() + np.log(cb).sum()
                        - np.log(sv).sum())
            safe = 0.0 if np.isinf(raw) else raw
            per_sample[b] = safe / max(int(tl[b]), 1)
    return np.asarray(per_sample.mean(), dtype=np.float32)


# revision 18
# speedup vs baseline: 1.0916x; 1.0092x over previous
"""CTC loss kernel for Trainium2 (8 NeuronCores, batch-parallel).

Strategy (v3)
-------------
Batch B=64 sharded 8 samples/core. pred is cast to fp8(e4m3) on the HOST, so
the device streams 8.5 MB/core instead of 34 MB (HBM read is the roofline:
measured ~125-170 GB/s/core regardless of queue mix, so fewer bytes win).
Per core, ten [128, 6625] tiles (partition p = b*16 + r) stream through:

  1. DMA in, alternating the two HWDGE queues (sync/scalar); the first two
     tile PAIRS are split into sample-halves issued on BOTH queues in
     parallel so the DP pipeline starts ~35us earlier. Tiles 0-4 load
     time-ascending; tiles 9-5 load TIME-REVERSED via a negative-stride
     DRAM access pattern (r = 15 - t_local).
  2. ScalarE: Exp fp8->f32 with fused per-row accumulate. The softmax
     denominators s are NOT applied on device - they are written out and
     folded back on the host (log-domain, f64). A 31-column zero pad is
     memset next to the exp output.
  3. GPSIMD ap_gather picks 64 columns per row: the extended-label class
     columns, with per-sample band masking (state > 2*target_len) baked in
     by pointing masked states at the zero pad column. Tiles 0-4 gather
     forward states; tiles 9-5 gather MIRRORED states for the backward DP.
  4. Regroup (b*16+r, s) -> (b, r, s) via DRAM scratch (cast to bf16 on the
     way out) into a [16, 16, 64] bf16 tile: rows 0-7 = forward p, rows
     8-15 = backward p-tilde.

The CTC recursion is split in half: alpha runs forward over t=0..79 (rows
0-7) while beta runs backward over t=159..80 (rows 8-15, state-mirrored so
its shifts match alpha's). Both advance in the SAME VectorE instructions
([16, 51] bf16 tiles), halving the serial chain to 80 steps:

  new[s] = (cur[s] + cur[s-1] + skip[s]*cur[s-2]) * p[t, s]

Renormalization every 8 steps is POSTPONED: the sum is computed on GPSIMD
(off the VectorE critical path) and applied as a single tensor-scalar
divide fused after the next step's multiply; the final step renormalizes
in-chain. Host epilogue: L = sum_s alpha_79[s] * (beta_80[s] + beta_80[s+1]
+ skip[s+2]*beta_80[s+2]), NLL = -(log L + sum log c - sum_t log s_t),
zero-infinity, length division, batch mean (f64, 64 samples).
"""

from contextlib import ExitStack

import numpy as np
import ml_dtypes

import concourse.bass as bass
import concourse.tile as tile
from concourse import bacc, mybir
from concourse.ap import AP
from concourse.bass_utils import run_bass_kernel_spmd

N_CORES = 8
B = 64
T = 160
C = 6625
L = 25
S = 2 * L + 1           # 51 extended states
BPC = B // N_CORES      # 8 samples per core
TBLK = 16               # time steps per streamed tile
NBLK = T // TBLK        # 10 tiles per core
GC = 64                 # gather columns (51 states padded to 64)
CP = C + 31             # exp tile free size incl. zero pad columns
ZCOL = C                # index of the (memset) zero column
ROWS = 16               # 0-7 fwd alpha, 8-15 bwd beta (mirrored)
NPH = 5                 # DP phases (80 steps / 16)
NNORM = 10              # renorm sums at j = 7, 15, ..., 79
NSPLIT = 0              # ORDER positions whose DMA is split across both queues
ORDER = [0, 9, 1, 8, 2, 7, 3, 6, 4, 5]

FP = mybir.dt.float32
BF = mybir.dt.bfloat16
F8 = mybir.dt.float8e4
ADD = mybir.AluOpType.add
DIV = mybir.AluOpType.divide


def build_nc() -> bass.Bass:
    nc = bacc.Bacc("TRN2", target_bir_lowering=False, debug=False,
                   num_devices=N_CORES)
    pred = nc.dram_tensor("pred", [BPC, T, C], F8, kind="ExternalInput")
    idxf = nc.dram_tensor("idxf", [128, GC // 16], mybir.dt.int16,
                          kind="ExternalInput")
    idxb = nc.dram_tensor("idxb", [128, GC // 16], mybir.dt.int16,
                          kind="ExternalInput")
    maskv = nc.dram_tensor("maskv", [ROWS, GC], BF, kind="ExternalInput")
    initm = nc.dram_tensor("initm", [ROWS, GC], BF, kind="ExternalInput")
    out_alpha = nc.dram_tensor("out_alpha", [ROWS, GC], BF, kind="ExternalOutput")
    out_c = nc.dram_tensor("out_c", [ROWS, NNORM], FP, kind="ExternalOutput")
    out_s = nc.dram_tensor("out_s", [128, NBLK], FP, kind="ExternalOutput")

    with tile.TileContext(nc) as tc, ExitStack() as ctx:
        pred_pool = ctx.enter_context(tc.tile_pool(name="pred_pool", bufs=3))
        exp_pool = ctx.enter_context(tc.tile_pool(name="exp_pool", bufs=3))
        small = ctx.enter_context(tc.tile_pool(name="small", bufs=3))
        pdp_pool = ctx.enter_context(tc.tile_pool(name="pdp_pool", bufs=3))
        dram_pool = ctx.enter_context(
            tc.tile_pool(name="pscr_pool", bufs=4, space="DRAM"))

        def single(shape, dtype, name):
            t, free = tc.tile(shape, dtype, name=name)
            ctx.callback(free)
            return t

        idxf_sb = single([128, GC // 16], mybir.dt.int16, "idxf_sb")
        idxb_sb = single([128, GC // 16], mybir.dt.int16, "idxb_sb")
        maskv_sb = single([ROWS, GC], BF, "maskv_sb")
        initm_sb = single([ROWS, GC], BF, "initm_sb")
        # ping/pong alpha with 2 guard columns each: ping states at 2..52,
        # pong states at 66..116; guards stay zero forever.
        alpha = single([ROWS, 128], BF, "alpha")
        cbuf = single([ROWS, NNORM], FP, "cbuf")
        rcn = single([ROWS, 1], FP, "rcn")
        sacc = single([128, NBLK], FP, "sacc")

        # constants go on the gpsimd queue so the first pred DMA leads sync
        nc.gpsimd.dma_start(out=idxf_sb[:, :], in_=idxf[:, :])
        nc.gpsimd.dma_start(out=idxb_sb[:, :], in_=idxb[:, :])
        nc.gpsimd.dma_start(out=maskv_sb[:, :], in_=maskv[:, :])
        nc.gpsimd.dma_start(out=initm_sb[:, :], in_=initm[:, :])
        nc.vector.memset(alpha[:, :], 0.0)

        def preprocess(k, i):
            """Stream exp-tile k (ORDER position i); returns its DRAM scratch."""
            pt = pred_pool.tile([128, C], F8, tag="pt")
            if i < NSPLIT:
                # split across BOTH queues in parallel for a fast ramp-up
                for h, q in enumerate((nc.sync, nc.scalar)):
                    rows = slice(64 * h, 64 * h + 64)
                    boff = 4 * h * T * C
                    if k < NPH:
                        q.dma_start(
                            out=pt[rows, :],
                            in_=pred[4 * h:4 * h + 4,
                                     k * TBLK:(k + 1) * TBLK, :],
                            max_dma_last_dim=3328)
                    else:
                        q.dma_start(
                            out=pt[rows, :],
                            in_=AP(pred, boff + (k * TBLK + TBLK - 1) * C,
                                   [[T * C, 4], [-C, TBLK], [1, C]]),
                            max_dma_last_dim=3328)
            else:
                q = nc.sync if i % 2 == 0 else nc.scalar
                if k < NPH:
                    q.dma_start(out=pt[:, :],
                                in_=pred[:, k * TBLK:(k + 1) * TBLK, :])
                else:
                    # time-reversed load: partition r = b*16 + (15 - t_local)
                    q.dma_start(out=pt[:, :],
                                in_=AP(pred, (k * TBLK + TBLK - 1) * C,
                                       [[T * C, BPC], [-C, TBLK], [1, C]]))
            et = exp_pool.tile([128, CP], FP, tag="et")
            nc.gpsimd.memset(et[:, C:CP], 0.0)
            nc.scalar.activation(
                out=et[:, 0:C], in_=pt[:, :],
                func=mybir.ActivationFunctionType.Exp,
                accum_out=sacc[:, i:i + 1],
            )
            g = small.tile([128, GC], FP, tag="g")
            nc.gpsimd.ap_gather(
                g[:, :], et[:, 0:CP],
                idxf_sb[:, :] if k < NPH else idxb_sb[:, :],
                channels=128, num_elems=CP, d=1, num_idxs=GC,
            )
            # cast f32 -> bf16 on the SWDGE write leg
            pscr = dram_pool.tile([BPC, TBLK, GC], BF, name=f"pscr{i}")
            nc.gpsimd.dma_start(out=pscr[:, :, :], in_=g[:, :])
            return pscr

        PING, PONG = 0, 64
        jn = 0
        for m in range(NPH):
            pscr_f = preprocess(m, 2 * m)
            pscr_b = preprocess(NBLK - 1 - m, 2 * m + 1)
            pdp = pdp_pool.tile([ROWS, TBLK, GC], BF, tag="pdp")
            nc.gpsimd.dma_start(out=pdp[0:BPC, :, :], in_=pscr_f[:, :, :])
            nc.gpsimd.dma_start(out=pdp[BPC:2 * BPC, :, :], in_=pscr_b[:, :, :])
            for ti in range(TBLK):
                j = m * TBLK + ti
                if j == 0:
                    nc.vector.tensor_mul(
                        alpha[:, PING + 2:PING + 2 + S],
                        pdp[:, 0, 0:S], initm_sb[:, 0:S],
                    )
                    continue
                src = PING if j % 2 == 1 else PONG
                dst = PONG if j % 2 == 1 else PING
                vt = small.tile([ROWS, S], BF, tag="vt")
                nc.vector.tensor_mul(
                    vt[:, :], alpha[:, src:src + S], maskv_sb[:, 0:S]
                )
                ut = small.tile([ROWS, S], BF, tag="ut")
                nc.vector.tensor_add(
                    ut[:, :], alpha[:, src + 2:src + 2 + S],
                    alpha[:, src + 1:src + 1 + S],
                )
                nc.vector.tensor_add(ut[:, :], ut[:, :], vt[:, :])
                adst = alpha[:, dst + 2:dst + 2 + S]
                nc.vector.tensor_mul(adst, ut[:, :], pdp[:, ti, 0:S])
                if j % 8 == 7:
                    nc.vector.tensor_reduce(
                        out=cbuf[:, jn:jn + 1], in_=adst,
                        axis=mybir.AxisListType.X, op=ADD,
                    )
                    nc.vector.reciprocal(rcn[:, :], cbuf[:, jn:jn + 1])
                    nc.vector.tensor_scalar_mul(adst, adst, rcn[:, 0:1])
                    jn += 1

        assert jn == NNORM
        # final state after j=79 (odd) lives in the PONG half
        nc.sync.dma_start(out=out_alpha[:, :], in_=alpha[:, PONG:PONG + GC])
        nc.sync.dma_start(out=out_c[:, :], in_=cbuf[:, :])
        nc.sync.dma_start(out=out_s[:, :], in_=sacc[:, :])
    nc.compile()
    return nc


_CACHE: dict = {}


def _get_nc() -> bass.Bass:
    if "nc" not in _CACHE:
        _CACHE["nc"] = build_nc()
    return _CACHE["nc"]


LAST_RESULTS = None


def kernel(pred, targets, targets_lengths) -> np.ndarray:
    global LAST_RESULTS
    pred = np.ascontiguousarray(np.asarray(pred, dtype=np.float32))
    targets = np.asarray(targets).astype(np.int64)
    tl = np.asarray(targets_lengths).astype(np.int64)
    assert pred.shape == (B, T, C), pred.shape
    assert targets.shape == (B, L)

    # host prep: extended labels, skip masks, gather index tables
    ext = np.zeros((B, S), dtype=np.int64)
    ext[:, 1::2] = targets
    skip = np.zeros((B, S), dtype=np.float32)
    skip[:, 2:] = ((ext[:, 2:] != 0) & (ext[:, 2:] != ext[:, :-2])).astype(np.float32)

    pred8 = pred.astype(ml_dtypes.float8_e4m3fn)

    in_maps = []
    for c in range(N_CORES):
        lo = c * BPC
        idxf_h = np.full((128, GC // 16), ZCOL, dtype=np.int16)
        idxb_h = np.full((128, GC // 16), ZCOL, dtype=np.int16)
        mv = np.zeros((ROWS, GC), dtype=np.float32)
        im = np.zeros((ROWS, GC), dtype=np.float32)
        for g in range(BPC):
            b = lo + g
            band = 2 * int(tl[b])           # reachable states: 0..band
            for j in range(S):
                if j <= band:
                    idxf_h[16 * g + (j % 16), j // 16] = ext[b, j]
                sm = S - 1 - j              # mirrored state for the bwd DP
                if sm <= band:
                    idxb_h[16 * g + (j % 16), j // 16] = ext[b, sm]
            mv[g, :S] = skip[b]
            # mirrored skip for beta: m~[s~] = skip[52 - s~] for s~ in [2, 50]
            for st in range(2, S):
                mv[BPC + g, st] = skip[b, 52 - st]
            im[g, 0:2] = 1.0
            im[BPC + g, (S - 1) - band] = 1.0
            im[BPC + g, S - band] = 1.0
        in_maps.append({
            "pred": np.ascontiguousarray(pred8[lo:lo + BPC]),
            "idxf": idxf_h,
            "idxb": idxb_h,
            "maskv": mv.astype(ml_dtypes.bfloat16),
            "initm": im.astype(ml_dtypes.bfloat16),
        })

    nc = _get_nc()
    LAST_RESULTS = run_bass_kernel_spmd(nc, in_maps, core_ids=list(range(N_CORES)))
    results = LAST_RESULTS.results

    # host epilogue (f64, 64 samples)
    per_sample = np.zeros(B, dtype=np.float64)
    for c in range(N_CORES):
        oa = np.asarray(results[c]["out_alpha"]).astype(np.float64)   # [16, 64]
        oc = results[c]["out_c"].astype(np.float64)                   # [16, 10]
        os_ = results[c]["out_s"].astype(np.float64)                  # [128, 10]
        for g in range(BPC):
            b = c * BPC + g
            a = oa[g, 2:2 + S]                    # alpha_79[s]
            beta = oa[BPC + g, 2:2 + S][::-1]     # beta_80[s] (un-mirrored)
            gam = beta.copy()
            gam[:-1] += beta[1:]
            gam[:-2] += skip[b, 2:].astype(np.float64) * beta[2:]
            dot = float(np.dot(a, gam))
            cf = oc[g]
            cb = oc[BPC + g]
            sv = os_[16 * g:16 * (g + 1), :].reshape(-1)   # all 160 denominators
            if (dot <= 0.0 or not np.isfinite(dot)
                    or np.any(cf <= 0.0) or np.any(cb <= 0.0)
                    or np.any(sv <= 0.0) or not np.all(np.isfinite(cf))
                    or not np.all(np.isfinite(cb))):
                raw = np.inf
            else:
                raw = -(np.log(dot) + np.log(cf).sum() + np.log(cb).sum()
                        - np.log(sv).sum())
            safe = 0.0 if np.isinf(raw) else raw
            per_sample[b] = safe / max(int(tl[b]), 1)
    return np.asarray(per_sample.mean(), dtype=np.float32)
